# revision 1
# baseline (speedup 1.0000x reference)
"""Trainium2 Bass kernel for the QRNN-style recommender model.

Model (per batch row b):
  emb = item_emb[seq]                          # [T=16, D=256]
  conv_out[l,t,c] = sum_{m<=l} emb[t-m] @ W[l,m,c,:] + conv_b[l,c]   (L=16 causal convs)
  f = sigmoid(relu(conv_out))                  # forget gates
  h = fo-pool chain applied 3x over t (QRNN), x0 = emb
  o = sum over (l, t) of h                     # [D]
  z = [o, user_emb[user]] @ fc1_w.T + fc1_b    # [D]
  res[n] = W2[item[n]] . z + b2[item[n]]       # [N_TGT=32]

Sharding: data-parallel over batch B=512 across 8 cores (64 rows each);
all parameters/tables replicated; embedding gathers run on-device via
indirect DMA.

Per-core device layout:
  embT[kc][d(128), b(64), tpad(31)]  time-padded transposed gathered emb
  conv: psum[c(128), (b,t)(1024)] accumulated over (m, kc) with shifted
        time windows of embT; f32r matmuls (full-rate for N=512)
  gates: ACT relu(z+b) -> r; f = sigmoid(r); g = sigmoid(-r) = 1-f
  fo-pool: DVE tensor_tensor_scan (state = g*state + f*x) over a
        (b, 17)-slotted free dim; slot 0 per b is a reset (g=f*x=0)
  head: fc1 via PE, per-row dot with gathered W2 rows via DVE mul +
        ones-vector PE partition-reduction.
"""
import os
import numpy as np

import concourse.bass as bass
import concourse.mybir as mybir
import concourse.tile as tile
from concourse import bacc
from concourse.masks import make_identity

F32 = mybir.dt.float32
F32R = mybir.dt.float32r
BF16 = mybir.dt.bfloat16
I32 = mybir.dt.int32
AF = mybir.ActivationFunctionType
ALU = mybir.AluOpType

# model dims (hardcoded per problem spec)
N_CORES = 8
B = 512
BC = B // N_CORES          # 64 rows per core
T = 16
L = 16
D = 256
N_TGT = 32
N_ITEMS = 200000
N_USERS = 100000
N_L = 3                    # fo-pool chain depth
PAD = L - 1                # 15 zero columns of left time padding
TW = T + PAD               # 31
S = T + 1                  # 17 scan slots per b (slot 0 = reset)
TRI = [l * (l + 1) // 2 for l in range(L + 1)]  # block offsets for (l, m<=l)


def _build_kernel(nc, tc):
    seq8 = nc.dram_tensor("seq8", [8, 128], I32, kind="ExternalInput").ap()
    item16 = nc.dram_tensor("item16", [16, 128], I32, kind="ExternalInput").ap()
    useri = nc.dram_tensor("useri", [BC], I32, kind="ExternalInput").ap()
    item_emb = nc.dram_tensor("item_emb", [N_ITEMS, D], F32, kind="ExternalInput").ap()
    user_emb = nc.dram_tensor("user_emb", [N_USERS, D], F32, kind="ExternalInput").ap()
    w2tab = nc.dram_tensor("w2tab", [N_ITEMS, D], F32, kind="ExternalInput").ap()
    wt = nc.dram_tensor("wt", [TRI[L], D, D], BF16, kind="ExternalInput").ap()
    convb = nc.dram_tensor("convb", [128, 2, L], F32, kind="ExternalInput").ap()
    fc1wt = nc.dram_tensor("fc1wt", [2 * D, D], F32, kind="ExternalInput").ap()
    fc1b = nc.dram_tensor("fc1b", [128, 2], F32, kind="ExternalInput").ap()
    res = nc.dram_tensor("res", [BC, N_TGT], F32, kind="ExternalOutput").ap()

    import contextlib
    ctx = contextlib.ExitStack()
    with ctx:
        perm = ctx.enter_context(tc.tile_pool(name="perm", bufs=1))
        idxp = ctx.enter_context(tc.tile_pool(name="idxp", bufs=2))
        gath = ctx.enter_context(tc.tile_pool(name="gath", bufs=4))
        wpool = ctx.enter_context(tc.tile_pool(name="wpool", bufs=8))
        rp = ctx.enter_context(tc.tile_pool(name="rp", bufs=6))
        fg = ctx.enter_context(tc.tile_pool(name="fg", bufs=5))
        tt = ctx.enter_context(tc.tile_pool(name="tt", bufs=5))
        small = ctx.enter_context(tc.tile_pool(name="small", bufs=2))
        cps = ctx.enter_context(tc.tile_pool(name="cps", bufs=6, space="PSUM"))
        tps = ctx.enter_context(tc.tile_pool(name="tps", bufs=2, space="PSUM"))

        ident = perm.tile([128, 128], F32, tag="ident")
        make_identity(nc, ident)

        # ---- phase A: gather seq embeddings, build embT[kc] = [128, 64, 31]
        embT = [perm.tile([128, BC, TW], F32, tag=f"embT{kc}", name=f"embT{kc}") for kc in (0, 1)]
        embTb = [perm.tile([128, TW, BC], BF16, tag=f"embTb{kc}", name=f"embTb{kc}") for kc in (0, 1)]
        for kc in (0, 1):
            nc.vector.memset(embT[kc][:, :, 0:PAD], 0.0)
            nc.gpsimd.memset(embTb[kc][:, 0:PAD, :], 0.0)
        for c in range(8):
            it = idxp.tile([128, 1], I32, tag="seqidx")
            nc.sync.dma_start(it[:], seq8[c, :, None])
            gt = gath.tile([128, D], F32, tag="embg")
            nc.gpsimd.indirect_dma_start(
                out=gt[:], out_offset=None, in_=item_emb[:],
                in_offset=bass.IndirectOffsetOnAxis(ap=it[:, :1], axis=0))
            for kc in (0, 1):
                tp = tps.tile([128, 128], F32, tag="tp")
                nc.tensor.transpose(tp[:], gt[:, kc * 128:(kc + 1) * 128], ident[:])
                nc.scalar.copy(embT[kc][:, 8 * c:8 * (c + 1), PAD:TW], tp[:])
                nc.scalar.copy(embTb[kc][:, PAD:TW, 8 * c:8 * (c + 1)].rearrange("p t b -> p b t"), tp[:])

        # ---- conv biases
        cb = perm.tile([128, 2, L], F32, tag="cb")
        nc.sync.dma_start(cb[:], convb[:])

        # ---- output accumulators o[c, b]
        oacc = [perm.tile([128, BC], F32, tag=f"oacc{cc}", name=f"oacc{cc}") for cc in (0, 1)]
        o3acc = [perm.tile([128, BC, S], F32, tag=f"o3acc{cc}", name=f"o3acc{cc}") for cc in (0, 1)]
        for cc in (0, 1):
            nc.vector.memset(o3acc[cc][:], 0.0)

        # user embedding -> uT chunks
        uidx = idxp.tile([BC, 1], I32, tag="uidx")
        nc.sync.dma_start(uidx[:], useri[:, None])
        ug = gath.tile([BC, D], F32, tag="ug")
        nc.gpsimd.indirect_dma_start(
            out=ug[:], out_offset=None, in_=user_emb[:],
            in_offset=bass.IndirectOffsetOnAxis(ap=uidx[:, :1], axis=0))
        catT = [oacc[0], oacc[1]]
        for kc in (0, 1):
            tp = tps.tile([128, 128], F32, tag="tp")
            nc.tensor.transpose(tp[:, :BC], ug[:, kc * 128:(kc + 1) * 128], ident[:BC, :BC])
            ut = small.tile([128, BC], F32, tag=f"ut{kc}")
            nc.any.tensor_copy(ut[:], tp[:, :BC])
            catT.append(ut)

        # W2 row gathers -> w2t[kc] = [128, 2048] (c on partitions, (b,n) free)
        w2t = [perm.tile([128, BC * N_TGT], F32, tag=f"w2t{kc}", name=f"w2t{kc}") for kc in (0, 1)]
        for ch in range(16):
            it = idxp.tile([128, 1], I32, tag="itemidx")
            nc.sync.dma_start(it[:], item16[ch, :, None])
            wg = gath.tile([128, D], F32, tag="w2g")
            nc.gpsimd.indirect_dma_start(
                out=wg[:], out_offset=None, in_=w2tab[:],
                in_offset=bass.IndirectOffsetOnAxis(ap=it[:, :1], axis=0))
            for kc in (0, 1):
                tp = tps.tile([128, 128], F32, tag="tp")
                nc.tensor.transpose(tp[:], wg[:, kc * 128:(kc + 1) * 128], ident[:])
                nc.scalar.copy(w2t[kc][:, 128 * ch:128 * (ch + 1)], tp[:])

        # ---- phase B: per-l conv + gates + triple fo-pool scan
        for l in range(L):
            wts = []
            for m in range(l + 1):
                w_t = wpool.tile([128, 2, D], BF16, tag="wt")
                nc.sync.dma_start(w_t[:], wt[TRI[l] + m].rearrange("(kc k) c -> k kc c", k=128))
                wts.append(w_t)
            pst = [[cps.tile([128, 512], F32, tag="cps", name=f"pst{l}_{i}_{h}")
                    for h in (0, 1)] for i in (0, 1)]
            for m in range(l + 1):
                for kc in (0, 1):
                    for cc in (0, 1):
                        lhs = wts[m][:, kc, cc * 128:(cc + 1) * 128]
                        for h in (0, 1):
                            # psum is t-major per half: col = 32*t + b. Taps with
                            # t < m are structurally zero -> write cols [32m, 512)
                            rhs = embTb[kc][:, PAD:PAD + T - m, 32 * h:32 * (h + 1)]
                            nc.tensor.matmul(
                                pst[cc][h][:, 32 * m:512],
                                lhsT=lhs, rhs=rhs,
                                start=(m == 0 and kc == 0),
                                stop=(m == l and kc == 1))
            fts, gts = [], []
            for cc in (0, 1):
                # r = relu(z + b);  f = sigmoid(r);  g = sigmoid(-r) = 1 - f
                f_t = fg.tile([128, BC, S], F32, tag="f", name=f"f{l}_{cc}")
                g_t = fg.tile([128, BC, S], F32, tag="g", name=f"g{l}_{cc}")
                nc.gpsimd.memset(f_t[:, :, 0:1], 0.0)
                nc.gpsimd.memset(g_t[:, :, 0:1], 0.0)
                for h in (0, 1):
                    r_t = rp.tile([128, 512], F32, tag="r")
                    nc.scalar.activation(r_t[:], pst[cc][h][:], AF.Relu,
                                         bias=cb[:, cc, l:l + 1], scale=1.0)
                    r3 = r_t[:].rearrange("p (t b) -> p t b", t=T)
                    f3 = f_t[:, 32 * h:32 * (h + 1), 1:S].rearrange("p b t -> p t b")
                    g3 = g_t[:, 32 * h:32 * (h + 1), 1:S].rearrange("p b t -> p t b")
                    nc.scalar.activation(f3, r3, AF.Sigmoid)
                    nc.scalar.activation(g3, r3, AF.Sigmoid, scale=-1.0)
                fts.append(f_t); gts.append(g_t)
            # interleave the two cc chains so Pool muls and DVE scans ping-pong
            curs = [None, None]
            for chain in range(N_L):
                fxs = [None, None]
                for cc in (0, 1):
                    fx = tt.tile([128, BC, S], F32, tag="fx", name=f"fx{l}_{cc}_{chain}")
                    xin = embT[cc][:, :, PAD - 1:TW] if chain == 0 else curs[cc][:]
                    nc.gpsimd.tensor_tensor(out=fx[:], in0=fts[cc][:], in1=xin, op=ALU.mult)
                    fxs[cc] = fx
                for cc in (0, 1):
                    hn = tt.tile([128, BC, S], F32, tag="hh", name=f"hh{l}_{cc}_{chain}")
                    nc.vector.tensor_tensor_scan(
                        out=hn[:].rearrange("p b t -> p (b t)"),
                        data0=gts[cc][:].rearrange("p b t -> p (b t)"),
                        data1=fxs[cc][:].rearrange("p b t -> p (b t)"),
                        initial=0.0, op0=ALU.mult, op1=ALU.add)
                    curs[cc] = hn
            for cc in (0, 1):
                nc.gpsimd.dma_start(o3acc[cc][:], curs[cc][:], accum_op=ALU.add)

        for cc in (0, 1):
            nc.vector.reduce_sum(oacc[cc][:], o3acc[cc][:], axis=mybir.AxisListType.X)

        # ---- phase C: head (gathers/transposes hoisted before conv)
        # z^T = fc1_w @ cat^T + b  -> [zc(2 chunks of 128), b(64)]
        f1w = perm.tile([128, 4, D], F32, tag="f1w")
        nc.sync.dma_start(f1w[:], fc1wt.rearrange("(kc k) c -> k kc c", k=128))
        f1b = perm.tile([128, 2], F32, tag="f1b")
        nc.sync.dma_start(f1b[:], fc1b[:])
        zT = []
        for cc in (0, 1):
            zp = tps.tile([128, BC], F32, tag="tp")
            for kc in range(4):
                nc.tensor.matmul(
                    zp[:], lhsT=f1w[:, kc, cc * 128:(cc + 1) * 128],
                    rhs=catT[kc][:],
                    start=(kc == 0), stop=(kc == 3))
            zt = small.tile([128, BC], F32, tag=f"zt{cc}")
            nc.scalar.activation(zt[:], zp[:], AF.Identity, bias=f1b[:, cc:cc + 1])
            zT.append(zt)

        # res[b,n] = sum_c w2t[c,(b,n)] * z[c,b]  (mul + ones-matmul partition sum)
        for kc in (0, 1):
            nc.gpsimd.tensor_tensor(
                out=w2t[kc][:].rearrange("p (b n) -> p b n", n=N_TGT),
                in0=w2t[kc][:].rearrange("p (b n) -> p b n", n=N_TGT),
                in1=zT[kc][:, :, None].to_broadcast((128, BC, N_TGT)),
                op=ALU.mult)
        ones = small.tile([128, 1], F32, tag="ones")
        nc.vector.memset(ones[:], 1.0)
        res_sb = small.tile([1, BC * N_TGT], F32, tag="ressb")
        for j in range(4):
            rj = tps.tile([1, 512], F32, tag="tp")
            for kc in (0, 1):
                nc.tensor.matmul(rj[:], lhsT=ones[:],
                                 rhs=w2t[kc][:, 512 * j:512 * (j + 1)],
                                 start=(kc == 0), stop=(kc == 1))
            nc.any.tensor_copy(res_sb[:, 512 * j:512 * (j + 1)], rj[:])
        nc.sync.dma_start(res.rearrange("b n -> (b n)")[None, :], res_sb[:])


_CACHED_NC = None


def build_nc():
    global _CACHED_NC
    if _CACHED_NC is not None:
        return _CACHED_NC
    nc = bacc.Bacc("TRN2", debug=False, enable_asserts=False)
    with tile.TileContext(nc) as tc:
        _build_kernel(nc, tc)
    nc.compile()
    _CACHED_NC = nc
    return nc


def make_in_maps(seq_var, user_var, item_var, item_emb, user_emb, conv_w,
                 conv_b, fc1_w, fc1_b, W2, b2):
    seq_var = np.asarray(seq_var).astype(np.int32)
    user_var = np.asarray(user_var).astype(np.int32)
    item_var = np.asarray(item_var).astype(np.int32)
    item_emb = np.ascontiguousarray(np.asarray(item_emb, dtype=np.float32))
    user_emb = np.ascontiguousarray(np.asarray(user_emb, dtype=np.float32))
    W2 = np.ascontiguousarray(np.asarray(W2, dtype=np.float32))
    conv_w = np.asarray(conv_w, dtype=np.float32)
    conv_b = np.ascontiguousarray(np.asarray(conv_b, dtype=np.float32))
    fc1_w = np.asarray(fc1_w, dtype=np.float32)
    fc1_b = np.ascontiguousarray(np.asarray(fc1_b, dtype=np.float32))

    # pack conv weights: block (l, m<=l) at TRI[l]+m = conv_w[l, m].T  ([d, c]), bf16
    import ml_dtypes
    wt_pack = np.empty((TRI[L], D, D), ml_dtypes.bfloat16)
    for l in range(L):
        for m in range(l + 1):
            wt_pack[TRI[l] + m] = conv_w[l, m].T.astype(ml_dtypes.bfloat16)
    fc1wt = np.ascontiguousarray(fc1_w.T)
    # convb_pack[c, cc, l] = conv_b[l, cc*128 + c];  fc1b_pack[c, cc] = fc1_b[cc*128+c]
    convb_pack = np.ascontiguousarray(conv_b.reshape(L, 2, 128).transpose(2, 1, 0))
    fc1b_pack = np.ascontiguousarray(fc1_b.reshape(2, 128).T)

    in_maps = []
    for c in range(N_CORES):
        sl = slice(c * BC, (c + 1) * BC)
        in_maps.append({
            "seq8": np.ascontiguousarray(seq_var[sl].reshape(8, 128)),
            "item16": np.ascontiguousarray(item_var[sl].reshape(16, 128)),
            "useri": np.ascontiguousarray(user_var[sl]),
            "item_emb": item_emb,
            "user_emb": user_emb,
            "w2tab": W2,
            "wt": wt_pack,
            "convb": convb_pack,
            "fc1wt": fc1wt,
            "fc1b": fc1b_pack,
        })
    return in_maps


def kernel(seq_var, user_var, item_var, item_emb, user_emb, conv_w, conv_b,
           fc1_w, fc1_b, W2, b2, _trace=False):
    from concourse import bass_utils
    nc = build_nc()
    in_maps = make_in_maps(seq_var, user_var, item_var, item_emb, user_emb,
                           conv_w, conv_b, fc1_w, fc1_b, W2, b2)
    r = bass_utils.run_bass_kernel_spmd(
        nc, in_maps, core_ids=list(range(N_CORES)), trace=_trace)
    out = np.concatenate([r.results[c]["res"] for c in range(N_CORES)], axis=0)
    b2 = np.asarray(b2, dtype=np.float32)
    item_var = np.asarray(item_var)
    out = out + b2[item_var][..., 0]
    if _trace:
        return out.astype(np.float32), r
    return out.astype(np.float32)



# revision 5
# speedup vs baseline: 1.7193x; 1.7193x over previous
"""Trainium2 Bass kernel for the QRNN-style recommender model.

Model (per batch row b):
  emb = item_emb[seq]                          # [T=16, D=256]
  conv_out[l,t,c] = sum_{m<=l} emb[t-m] @ W[l,m,c,:] + conv_b[l,c]   (L=16 causal convs)
  f = sigmoid(relu(conv_out)) = max(sigmoid(conv_out), 0.5)
  h = fo-pool chain applied 3x over t (QRNN), x0 = emb
  o = sum over (l, t) of h                     # [D]
  z = [o, user_emb[user]] @ fc1_w.T + fc1_b    # [D]
  res[n] = W2[item[n]] . z + b2[item[n]]       # [N_TGT=32]

Sharding: data-parallel over batch B=512 across 8 cores (64 rows each);
all parameters/tables replicated; embedding gathers via indirect DMA.

Per-core layout (v2 — unrolled fo-pool, no tensor_tensor_scan):
  embTb[kc][d(128), t(16), b(64)]  bf16 transposed gathered emb (conv rhs)
  X0b[d(128), t, cc(2), b]         fp16 emb for the fo-pool x0
  conv: per (l, cc) one psum tile [c(128), bank(2), t8, b64] (t-split
        banks, col = 64*t + b), accumulated over (m, kc) bf16 matmuls
  gates: one ACT sigmoid per (l, cc) -> F[d, t, cc, l8, b] fp16;
        DVE tensor_scalar max(., 0.5) clamp in place
  fo-pool: 2 rounds of 8 l's; per round 3 chains x 16 unrolled steps of
        fp16 DVE tensor_tensor (d = x - h; e = f*d; h = h + e), each op
        [128, 2*8*64] at DVE 2x rate; x of chain 0 broadcasts X0b over l
  o: in-place binary-tree adds over t then l, accumulated into oacc f32
  head: fc1 via PE, per-row dot with gathered W2 rows via DVE mul +
        ones-vector PE partition-reduction.
"""
import os
import numpy as np

import concourse.bass as bass
import concourse.mybir as mybir
import concourse.tile as tile
from concourse import bacc
from concourse.masks import make_identity

F32 = mybir.dt.float32
BF16 = mybir.dt.bfloat16
F16 = mybir.dt.float16
I32 = mybir.dt.int32
AF = mybir.ActivationFunctionType
ALU = mybir.AluOpType

# model dims (hardcoded per problem spec)
N_CORES = 8
B = 512
BC = B // N_CORES          # 64 rows per core
T = 16
L = 16
L8 = 8                     # l's per round
D = 256
N_TGT = 32
N_ITEMS = 200000
N_USERS = 100000
N_L = 3                    # fo-pool chain depth
TRI = [l * (l + 1) // 2 for l in range(L + 1)]  # block offsets for (l, m<=l)


def _build_kernel(nc, tc):
    seq8 = nc.dram_tensor("seq8", [8, 128], I32, kind="ExternalInput").ap()
    item16 = nc.dram_tensor("item16", [16, 128], I32, kind="ExternalInput").ap()
    useri = nc.dram_tensor("useri", [BC], I32, kind="ExternalInput").ap()
    item_emb = nc.dram_tensor("item_emb", [N_ITEMS, D], F32, kind="ExternalInput").ap()
    user_emb = nc.dram_tensor("user_emb", [N_USERS, D], F32, kind="ExternalInput").ap()
    w2tab = nc.dram_tensor("w2tab", [N_ITEMS, D], F32, kind="ExternalInput").ap()
    wt = nc.dram_tensor("wt", [TRI[L], D, D], BF16, kind="ExternalInput").ap()
    convb = nc.dram_tensor("convb", [128, 2, L], F32, kind="ExternalInput").ap()
    fc1wt = nc.dram_tensor("fc1wt", [2 * D, D], F32, kind="ExternalInput").ap()
    fc1b = nc.dram_tensor("fc1b", [128, 2], F32, kind="ExternalInput").ap()
    res = nc.dram_tensor("res", [BC, N_TGT], F32, kind="ExternalOutput").ap()

    import contextlib
    ctx = contextlib.ExitStack()
    with ctx:
        perm = ctx.enter_context(tc.tile_pool(name="perm", bufs=1))
        idxp = ctx.enter_context(tc.tile_pool(name="idxp", bufs=2))
        gath = ctx.enter_context(tc.tile_pool(name="gath", bufs=4))
        wpool = ctx.enter_context(tc.tile_pool(name="wpool", bufs=8))
        st = ctx.enter_context(tc.tile_pool(name="st", bufs=2))
        small = ctx.enter_context(tc.tile_pool(name="small", bufs=2))
        cps = ctx.enter_context(tc.tile_pool(name="cps", bufs=3, space="PSUM"))
        tps = ctx.enter_context(tc.tile_pool(name="tps", bufs=2, space="PSUM"))

        ident = perm.tile([128, 128], F32, tag="ident")
        make_identity(nc, ident)

        cb = perm.tile([128, 2, L], F32, tag="cb")
        nc.sync.dma_start(cb[:], convb[:])
        f1w = perm.tile([128, 4, D], F32, tag="f1w")
        nc.sync.dma_start(f1w[:], fc1wt.rearrange("(kc k) c -> k kc c", k=128))
        f1b = perm.tile([128, 2], F32, tag="f1b")
        nc.sync.dma_start(f1b[:], fc1b[:])

        # ---- phase A: gather seq embeddings -> embTb (bf16), X0b (fp16)
        embTb = [perm.tile([128, T, BC], BF16, tag=f"embTb{kc}", name=f"embTb{kc}")
                 for kc in (0, 1)]
        X0b = perm.tile([128, T, 2, BC], F16, tag="x0b")
        for c in range(8):
            it = idxp.tile([128, 1], I32, tag="seqidx")
            nc.sync.dma_start(it[:], seq8[c, :, None])
            gt = gath.tile([128, D], F32, tag="embg")
            nc.gpsimd.indirect_dma_start(
                out=gt[:], out_offset=None, in_=item_emb[:],
                in_offset=bass.IndirectOffsetOnAxis(ap=it[:, :1], axis=0))
            for kc in (0, 1):
                tp = tps.tile([128, 128], F32, tag="tp")
                nc.tensor.transpose(tp[:], gt[:, kc * 128:(kc + 1) * 128], ident[:])
                tpv = tp[:].rearrange("p (b t) -> p t b", b=8)
                nc.scalar.copy(embTb[kc][:, :, 8 * c:8 * (c + 1)], tpv)
                nc.scalar.copy(X0b[:, :, kc, 8 * c:8 * (c + 1)], tpv)

        # ---- big fp16 state tiles
        Ft = [perm.tile([128, T, 2, L8, BC], F16, tag=f"ft{r}", name=f"ft{r}")
              for r in (0, 1)]
        HA = perm.tile([128, T, 2, L8, BC], F16, tag="ha")
        HB = perm.tile([128, T, 2, L8, BC], F16, tag="hb")
        oacc = [perm.tile([128, BC], F32, tag=f"oacc{cc}", name=f"oacc{cc}")
                for cc in (0, 1)]

        def conv_gates(r):
            """conv + sigmoid gates for l in [8r, 8r+8) -> Ft[r]."""
            for li in range(L8):
                gl = L8 * r + li
                wts = []
                for m in range(gl + 1):
                    w_t = wpool.tile([128, 2, D], BF16, tag="wt")
                    nc.sync.dma_start(
                        w_t[:], wt[TRI[gl] + m].rearrange("(kc k) c -> k kc c", k=128))
                    wts.append(w_t)
                for cc in (0, 1):
                    # psum [c(128), bank(2), t8, b64]; col = 64*t + b, t-split banks
                    ps = cps.tile([128, 2, 8, BC], F32, tag="cps")
                    m0max = min(gl, 7)
                    # bank 0: t in [0, 8)
                    for m in range(m0max + 1):
                        for kc in (0, 1):
                            nc.tensor.matmul(
                                ps[:, 0, m:8, :],
                                lhsT=wts[m][:, kc, cc * 128:(cc + 1) * 128],
                                rhs=embTb[kc][:, 0:8 - m, :],
                                start=(m == 0 and kc == 0),
                                stop=(m == m0max and kc == 1))
                    # bank 1: t in [8, 16)
                    for m in range(gl + 1):
                        for kc in (0, 1):
                            if m < 8:
                                out_ap = ps[:, 1, :, :]
                                rhs = embTb[kc][:, 8 - m:16 - m, :]
                            else:
                                out_ap = ps[:, 1, m - 8:8, :]
                                rhs = embTb[kc][:, 0:16 - m, :]
                            nc.tensor.matmul(
                                out_ap,
                                lhsT=wts[m][:, kc, cc * 128:(cc + 1) * 128],
                                rhs=rhs,
                                start=(m == 0 and kc == 0),
                                stop=(m == gl and kc == 1))
                    fsl = Ft[r][:, :, cc, li, :]
                    nc.scalar.activation(
                        fsl, ps[:].rearrange("p bk t b -> p (bk t) b"),
                        AF.Sigmoid, bias=cb[:, cc, gl:gl + 1], scale=1.0)
                    # f = sigmoid(relu(z)) = max(sigmoid(z), 0.5)
                    nc.vector.tensor_scalar_max(out=fsl, in0=fsl, scalar1=0.5)

        def fopool(r):
            """triple fo-pool over t for Ft[r]; o accumulated into oacc."""
            F = Ft[r]
            for ci, (xsrc, hout) in enumerate(((None, HA), (HA, HB), (HB, HA))):
                for t in range(T):
                    if xsrc is None:
                        xt = X0b[:, t, :, None, :].to_broadcast((128, 2, L8, BC))
                    else:
                        xt = xsrc[:, t]
                    ft = F[:, t]
                    if t == 0:
                        # h_0 = f_0 * x_0
                        nc.vector.tensor_tensor(
                            out=hout[:, 0], in0=ft, in1=xt, op=ALU.mult)
                    else:
                        # h_t = h_{t-1} + f_t * (x_t - h_{t-1})
                        d = st.tile([128, 2, L8, BC], F16, tag="std")
                        nc.vector.tensor_tensor(
                            out=d[:], in0=xt, in1=hout[:, t - 1], op=ALU.subtract)
                        e = st.tile([128, 2, L8, BC], F16, tag="ste")
                        nc.vector.tensor_tensor(
                            out=e[:], in0=ft, in1=d[:], op=ALU.mult)
                        nc.vector.tensor_tensor(
                            out=hout[:, t], in0=hout[:, t - 1], in1=e[:], op=ALU.add)
            # sum over t (in-place tree on chain-3 output = HA)
            w = T
            while w > 1:
                w //= 2
                nc.vector.tensor_tensor(
                    out=HA[:, 0:w], in0=HA[:, 0:w], in1=HA[:, w:2 * w], op=ALU.add)
            # sum over l (in-place tree on HA[:, 0])
            wl = L8
            while wl > 1:
                wl //= 2
                nc.vector.tensor_tensor(
                    out=HA[:, 0, :, 0:wl], in0=HA[:, 0, :, 0:wl],
                    in1=HA[:, 0, :, wl:2 * wl], op=ALU.add)
            for cc in (0, 1):
                if r == 0:
                    nc.scalar.copy(oacc[cc][:], HA[:, 0, cc, 0, :])
                else:
                    stmp = small.tile([128, BC], F32, tag="stmp")
                    nc.scalar.copy(stmp[:], HA[:, 0, cc, 0, :])
                    nc.vector.tensor_tensor(
                        out=oacc[cc][:], in0=oacc[cc][:], in1=stmp[:], op=ALU.add)

        conv_gates(0)
        fopool(0)
        conv_gates(1)
        fopool(1)

        # ---- user embedding -> catT chunks (PE transposes queue after conv)
        uidx = idxp.tile([BC, 1], I32, tag="uidx")
        nc.sync.dma_start(uidx[:], useri[:, None])
        ug = gath.tile([BC, D], F32, tag="ug", bufs=1)
        nc.gpsimd.indirect_dma_start(
            out=ug[:], out_offset=None, in_=user_emb[:],
            in_offset=bass.IndirectOffsetOnAxis(ap=uidx[:, :1], axis=0))
        catT = [oacc[0], oacc[1]]
        for kc in (0, 1):
            tp = tps.tile([128, 128], F32, tag="tp")
            nc.tensor.transpose(tp[:, :BC], ug[:, kc * 128:(kc + 1) * 128], ident[:BC, :BC])
            ut = small.tile([128, BC], F32, tag=f"ut{kc}")
            nc.any.tensor_copy(ut[:], tp[:, :BC])
            catT.append(ut)

        # W2 row gathers -> w2t[kc] = [128, 2048] (c on partitions, (b,n) free)
        w2t = [perm.tile([128, BC * N_TGT], F32, tag=f"w2t{kc}", name=f"w2t{kc}")
               for kc in (0, 1)]
        for ch in range(16):
            it = idxp.tile([128, 1], I32, tag="itemidx")
            nc.sync.dma_start(it[:], item16[ch, :, None])
            wg = gath.tile([128, D], F32, tag="w2g")
            nc.gpsimd.indirect_dma_start(
                out=wg[:], out_offset=None, in_=w2tab[:],
                in_offset=bass.IndirectOffsetOnAxis(ap=it[:, :1], axis=0))
            for kc in (0, 1):
                tp = tps.tile([128, 128], F32, tag="tp")
                nc.tensor.transpose(tp[:], wg[:, kc * 128:(kc + 1) * 128], ident[:])
                nc.scalar.copy(w2t[kc][:, 128 * ch:128 * (ch + 1)], tp[:])

        # ---- head: z^T = fc1_w @ cat^T + b  -> [zc(2 chunks of 128), b(64)]
        zT = []
        for cc in (0, 1):
            zp = tps.tile([128, BC], F32, tag="tp")
            for kcc in range(4):
                nc.tensor.matmul(
                    zp[:], lhsT=f1w[:, kcc, cc * 128:(cc + 1) * 128],
                    rhs=catT[kcc][:], start=(kcc == 0), stop=(kcc == 3))
            zt = small.tile([128, BC], F32, tag=f"zt{cc}")
            nc.scalar.activation(zt[:], zp[:], AF.Identity, bias=f1b[:, cc:cc + 1])
            zT.append(zt)

        # res[b,n] = sum_c w2t[c,(b,n)] * z[c,b]  (mul + ones-matmul partition sum)
        for kc in (0, 1):
            nc.vector.tensor_tensor(
                out=w2t[kc][:].rearrange("p (b n) -> p b n", n=N_TGT),
                in0=w2t[kc][:].rearrange("p (b n) -> p b n", n=N_TGT),
                in1=zT[kc][:, :, None].to_broadcast((128, BC, N_TGT)),
                op=ALU.mult)
        ones = small.tile([128, 1], F32, tag="ones")
        nc.vector.memset(ones[:], 1.0)
        res_sb = small.tile([1, BC * N_TGT], F32, tag="ressb", bufs=1)
        for j in range(4):
            rj = tps.tile([1, 512], F32, tag="tp")
            for kc in (0, 1):
                nc.tensor.matmul(rj[:], lhsT=ones[:],
                                 rhs=w2t[kc][:, 512 * j:512 * (j + 1)],
                                 start=(kc == 0), stop=(kc == 1))
            nc.any.tensor_copy(res_sb[:, 512 * j:512 * (j + 1)], rj[:])
        nc.sync.dma_start(res.rearrange("b n -> (b n)")[None, :], res_sb[:])


_CACHED_NC = None


def build_nc():
    global _CACHED_NC
    if _CACHED_NC is not None:
        return _CACHED_NC
    nc = bacc.Bacc("TRN2", debug=False, enable_asserts=False)
    with tile.TileContext(nc) as tc:
        _build_kernel(nc, tc)
    nc.compile()
    _CACHED_NC = nc
    return nc


def make_in_maps(seq_var, user_var, item_var, item_emb, user_emb, conv_w,
                 conv_b, fc1_w, fc1_b, W2, b2):
    seq_var = np.asarray(seq_var).astype(np.int32)
    user_var = np.asarray(user_var).astype(np.int32)
    item_var = np.asarray(item_var).astype(np.int32)
    item_emb = np.ascontiguousarray(np.asarray(item_emb, dtype=np.float32))
    user_emb = np.ascontiguousarray(np.asarray(user_emb, dtype=np.float32))
    W2 = np.ascontiguousarray(np.asarray(W2, dtype=np.float32))
    conv_w = np.asarray(conv_w, dtype=np.float32)
    conv_b = np.ascontiguousarray(np.asarray(conv_b, dtype=np.float32))
    fc1_w = np.asarray(fc1_w, dtype=np.float32)
    fc1_b = np.ascontiguousarray(np.asarray(fc1_b, dtype=np.float32))

    # pack conv weights: block (l, m<=l) at TRI[l]+m = conv_w[l, m].T  ([d, c]), bf16
    import ml_dtypes
    wt_pack = np.empty((TRI[L], D, D), ml_dtypes.bfloat16)
    for l in range(L):
        for m in range(l + 1):
            wt_pack[TRI[l] + m] = conv_w[l, m].T.astype(ml_dtypes.bfloat16)
    fc1wt = np.ascontiguousarray(fc1_w.T)
    # convb_pack[c, cc, l] = conv_b[l, cc*128 + c];  fc1b_pack[c, cc] = fc1_b[cc*128+c]
    convb_pack = np.ascontiguousarray(conv_b.reshape(L, 2, 128).transpose(2, 1, 0))
    fc1b_pack = np.ascontiguousarray(fc1_b.reshape(2, 128).T)

    in_maps = []
    for c in range(N_CORES):
        sl = slice(c * BC, (c + 1) * BC)
        in_maps.append({
            "seq8": np.ascontiguousarray(seq_var[sl].reshape(8, 128)),
            "item16": np.ascontiguousarray(item_var[sl].reshape(16, 128)),
            "useri": np.ascontiguousarray(user_var[sl]),
            "item_emb": item_emb,
            "user_emb": user_emb,
            "w2tab": W2,
            "wt": wt_pack,
            "convb": convb_pack,
            "fc1wt": fc1wt,
            "fc1b": fc1b_pack,
        })
    return in_maps


def kernel(seq_var, user_var, item_var, item_emb, user_emb, conv_w, conv_b,
           fc1_w, fc1_b, W2, b2, _trace=False):
    from concourse import bass_utils
    nc = build_nc()
    in_maps = make_in_maps(seq_var, user_var, item_var, item_emb, user_emb,
                           conv_w, conv_b, fc1_w, fc1_b, W2, b2)
    r = bass_utils.run_bass_kernel_spmd(
        nc, in_maps, core_ids=list(range(N_CORES)), trace=_trace)
    out = np.concatenate([r.results[c]["res"] for c in range(N_CORES)], axis=0)
    b2 = np.asarray(b2, dtype=np.float32)
    item_var = np.asarray(item_var)
    out = out + b2[item_var][..., 0]
    if _trace:
        return out.astype(np.float32), r
    return out.astype(np.float32)
